# revision 1
# baseline (speedup 1.0000x reference)
"""Trainium2 Bass kernel for nn_CESAR_24309514895978 (ragged_sequence).

Math (per batch b):
  m0 = (attention_masks==1)&(token_type_ids==0); m1 = (attention_masks==1)&(token_type_ids==1)
  score[i,j] = |emb_n[i] . emb_n[j]|   (L2-normalized embeddings)
  logits[i,j] = (emb@Wq.T+bq)[i] . (emb@Wk.T+bk)[j]
  cs[b] = sum_{valid ij} softmax_flat(logits | pair_mask)[i,j] * score[i,j]

Constant folding (host, once): the projections only enter through
  logits = embaug @ A_aug @ embaug.T,  embaug = [emb, 1],
  A_aug = [[Wq.T@Wk, Wq.T@bk], [bq.T@Wk, bq.bk]]   ((D+1)x(D+1))
so the per-batch device work is two chained matmuls instead of three.

Device, per batch (data-parallel: 2 batches per core x 8 cores, fp32r matmuls):
  - rsq[j] = sum_d emb[j,d]^2 (DVE squares+adds, one ones-column matmul);
    r = 1/sqrt (ACT Sqrt + DVE reciprocal); W2 = r row via gpsimd broadcast
  - P = A_aug @ embaug.T   (stage 1, 8 PSUM banks, db-outer accumulation;
    the u-row rides the PSUM->SBUF copy bias, c0 rides the prow copy bias)
  - L = embaug.T.T @ P + one K=3 matmul adding the -1e30 ragged-pair masks
    and the rank-1 prow term (rows: m0neg/ones/ones x ones/m1neg/prow)
  - M = masked max (DVE reduces + gpsimd partition_all_reduce)
  - E = exp(L - M) on ACT with accum_out -> Z partial sums
  - W partials = sum_j E * |G| * r_j  (G = gram matmul; |.| on ACT; stt fused)
Host: r_i scaling + final sums + W/Z division (tiny) + input layout/rounding.
"""
import numpy as np

import concourse.bass_isa as bass_isa
import concourse.tile as tile
from concourse import bacc, mybir
from concourse.bass_utils import run_bass_kernel_spmd

B, S, D = 16, 512, 1024
NCORES = 8
BPC = B // NCORES          # batches per core
NCH = D // 128             # 8 contraction chunks
NIC = S // 128             # 4 i-chunks
DA = D + 1                 # augmented dim
NEG = np.float32(-1e30)

F32 = mybir.dt.float32
F32R = mybir.dt.float32r
AFT = mybir.ActivationFunctionType
ALU = mybir.AluOpType
AX = mybir.AxisListType

PROFILE = False            # set True (e.g. from test.py) to capture NTFF profile
LAST_RESULTS = None        # BassKernelResults of the last run (for test.py)

_built = None


def _to_fp32r(x: np.ndarray) -> np.ndarray:
    """Round fp32 -> fp32r encoding (RNE to 11 explicit mantissa bits)."""
    u = np.ascontiguousarray(x, dtype=np.float32).view(np.uint32).astype(np.uint64)
    u = (u + 0x7FF + ((u >> 12) & 1)) & np.uint64(0xFFFFF000)
    return u.astype(np.uint32).view(np.float32)


def _build():
    global _built
    if _built is not None:
        return _built

    nc = bacc.Bacc("TRN2", target_bir_lowering=False, debug=False)

    embT_d = nc.dram_tensor("embT", [BPC, NCH, 128, S], F32R, kind="ExternalInput").ap()
    # AT[db, da] = A_aug[da, db]; rows 0..1023 in 8 chunks + row 1024 separate
    at_d = nc.dram_tensor("at", [DA, DA], F32R, kind="ExternalInput").ap()
    lrows_d = nc.dram_tensor("lrows", [BPC, 3, S], F32R, kind="ExternalInput").ap()
    rrows_d = nc.dram_tensor("rrows", [BPC, 2, S], F32R, kind="ExternalInput").ap()
    ucol_d = nc.dram_tensor("ucol", [128, NCH], F32, kind="ExternalInput").ap()
    c0_d = nc.dram_tensor("c0", [1, 1], F32, kind="ExternalInput").ap()
    ones_d = nc.dram_tensor("ones", [128, 1], F32R, kind="ExternalInput").ap()
    onesrow_d = nc.dram_tensor("onesrow", [1, S], F32R, kind="ExternalInput").ap()

    zw_d = nc.dram_tensor("zw", [BPC, 2, 128, NIC], F32, kind="ExternalOutput").ap()
    rout_d = nc.dram_tensor("rout", [BPC, S], F32, kind="ExternalOutput").ap()

    with tile.TileContext(nc) as tc:
        with (
            tc.tile_pool(name="apool", bufs=9) as apool,
            tc.tile_pool(name="spool", bufs=1) as spool,
            tc.tile_pool(name="epool", bufs=16) as epool,
            tc.tile_pool(name="sqpool", bufs=3) as sqpool,
            tc.tile_pool(name="paugpool", bufs=18) as paugpool,
            tc.tile_pool(name="w2pool", bufs=2) as w2pool,
            tc.tile_pool(name="gapool", bufs=2) as gapool,
            tc.tile_pool(name="gwpool", bufs=4) as gwpool,
            tc.tile_pool(name="Epool", bufs=2) as Epool,
            tc.tile_pool(name="scrpool", bufs=1) as scrpool,
            tc.tile_pool(name="tiny", bufs=2) as tiny,
            tc.tile_pool(name="lrpool", bufs=2) as lrpool,
            tc.tile_pool(name="ps", bufs=8, space="PSUM") as ps,
        ):
            # ---- first chunk pair goes absolutely first (PE start gate),
            # then the tiny loads, then the remaining big chunks interleaved.
            emb_all = [[None] * NCH for _ in range(BPC)]
            at_t = []
            t = apool.tile([128, DA], F32R, tag="a", name="at_0")
            nc.sync.dma_start(out=t[:], in_=at_d[0:128, :])
            at_t.append(t)
            t = epool.tile([128, S], F32R, tag="emb", name="emb0_0")
            nc.sync.dma_start(out=t[:], in_=embT_d[0, 0])
            emb_all[0][0] = t

            ones_col = spool.tile([128, 1], F32R, tag="ones_col")
            nc.sync.dma_start(out=ones_col[:], in_=ones_d)
            onesrow_t = spool.tile([1, S], F32R, tag="onesrow")
            nc.sync.dma_start(out=onesrow_t[:], in_=onesrow_d)
            ucol_t = spool.tile([128, NCH], F32, tag="ucol")
            nc.sync.dma_start(out=ucol_t[:], in_=ucol_d)
            c0_t = spool.tile([1, 1], F32, tag="c0")
            nc.sync.dma_start(out=c0_t[:], in_=c0_d)
            lr_all = []
            for b in range(BPC):
                lr_t = lrpool.tile([3, S], F32R, tag="lr", name=f"lr{b}")
                nc.sync.dma_start(out=lr_t[:], in_=lrows_d[b])
                lr_all.append(lr_t)

            for c in range(1, NCH):
                t = epool.tile([128, S], F32R, tag="emb", name=f"emb0_{c}")
                nc.sync.dma_start(out=t[:], in_=embT_d[0, c])
                emb_all[0][c] = t
                t = apool.tile([128, DA], F32R, tag="a", name=f"at_{c}")
                nc.sync.dma_start(out=t[:], in_=at_d[c * 128 : (c + 1) * 128, :])
                at_t.append(t)

            for b in range(BPC):
                # ---- load this batch's emb
                if b > 0:
                    for c in range(NCH):
                        t = epool.tile([128, S], F32R, tag="emb", name=f"emb{b}_{c}")
                        nc.sync.dma_start(out=t[:], in_=embT_d[b, c])
                        emb_all[b][c] = t
                emb_t = emb_all[b]
                lr_t = lr_all[b]

                # ---- stage 1: P = A_aug @ embaug.T  (db-outer over 8 banks);
                # the ones-row term (u) is folded into the copy bias below.
                st1 = [ps.tile([128, S], F32, tag="ps", name=f"st1_{b}_{da}")
                       for da in range(NCH)]
                prow_ps = ps.tile([1, S], F32, tag="ps")
                for db in range(NCH):
                    for da in range(NCH):
                        nc.tensor.matmul(st1[da][:],
                                         at_t[db][:, da * 128 : (da + 1) * 128],
                                         emb_t[db][:],
                                         start=(db == 0), stop=(db == NCH - 1))
                    # prow (P row 1024) rides the same chunk: 9 MMs per chunk
                    # pair matches the DMA arrival rate for batch 0
                    nc.tensor.matmul(prow_ps[:], at_t[db][:, D : D + 1],
                                     emb_t[db][:],
                                     start=(db == 0), stop=(db == NCH - 1))
                paug = []
                for da in range(NCH):
                    pt = paugpool.tile([128, S], F32R, tag="paug")
                    if da % 2 == 0:
                        nc.scalar.activation(out=pt[:], in_=st1[da][:],
                                             func=AFT.Identity,
                                             bias=ucol_t[:, da : da + 1], scale=1.0)
                    else:
                        nc.vector.tensor_scalar_add(pt[:], st1[da][:],
                                                    ucol_t[:, da : da + 1])
                    paug.append(pt)
                # P row 1024 (the bq-side rank-1 term); c0 folded into the bias
                prow = tiny.tile([1, S], F32R, tag="prow")
                nc.scalar.activation(out=prow[:], in_=prow_ps[:],
                                     func=AFT.Identity, bias=c0_t[:], scale=1.0)
                # rhs rows for the combined mask+prow matmul (K=3):
                # p0 = ones, p1 = m1neg (host), p2 = prow (device)
                rr3 = lrpool.tile([3, S], F32R, tag="rr3")
                nc.sync.dma_start(out=rr3[0:2, :], in_=rrows_d[b])
                nc.sync.dma_start(out=rr3[2:3, :], in_=prow[:])

                # ---- rsq / r / W2
                sqacc = sqpool.tile([128, S], F32R, tag="sqacc", bufs=2)
                sq0 = sqpool.tile([128, S], F32, tag="sq")
                nc.vector.tensor_mul(sq0[:], emb_t[0][:].bitcast(F32),
                                     emb_t[0][:].bitcast(F32))
                for c in range(1, NCH):
                    sq = sqpool.tile([128, S], F32, tag="sq")
                    nc.vector.tensor_mul(sq[:], emb_t[c][:].bitcast(F32),
                                         emb_t[c][:].bitcast(F32))
                    if c < NCH - 1:
                        nc.vector.tensor_add(sq0[:], sq0[:], sq[:])
                    else:
                        nc.vector.tensor_add(sqacc[:], sq0[:], sq[:])
                rsq_ps = ps.tile([1, S], F32, tag="ps")
                nc.tensor.matmul(rsq_ps[:], ones_col[:], sqacc[:],
                                 start=True, stop=True)
                s_row = tiny.tile([1, S], F32, tag="srow")
                nc.scalar.activation(out=s_row[:], in_=rsq_ps[:], func=AFT.Sqrt,
                                     bias=0.0, scale=1.0)
                r_row = tiny.tile([1, S], F32, tag="rrow")
                nc.vector.reciprocal(out=r_row[:], in_=s_row[:])
                nc.sync.dma_start(out=rout_d[b], in_=r_row[:])
                W2 = w2pool.tile([128, S], F32, tag="w2")
                nc.gpsimd.partition_broadcast(W2[:], r_row[0:1, :], channels=128)

                # ---- stage 2: L chunks + masks; per-chunk max
                mx = tiny.tile([128, NIC], F32, tag="mx")
                L_ps = []
                for ic in range(NIC):
                    Lp = ps.tile([128, S], F32, tag="ps", name=f"L_{b}_{ic}")
                    for da in range(NCH):
                        nc.tensor.matmul(Lp[:], emb_t[da][:, ic * 128 : (ic + 1) * 128],
                                         paug[da][:], start=(da == 0), stop=False)
                    nc.tensor.matmul(Lp[:], lr_t[:, ic * 128 : (ic + 1) * 128],
                                     rr3[:], start=False, stop=True)
                    nc.vector.reduce_max(mx[:, ic : ic + 1], Lp[:], axis=AX.X)
                    L_ps.append(Lp)

                # ---- global masked max -> -M in [128,1]
                par = tiny.tile([128, NIC], F32, tag="par")
                nc.gpsimd.partition_all_reduce(par[:], mx[:], channels=128,
                                               reduce_op=bass_isa.ReduceOp.max)
                negm128 = tiny.tile([128, 1], F32, tag="negm128")
                nc.vector.reduce_max(negm128[:], par[:], axis=AX.X, negate=True)

                # ---- gram chunks -> Gw = |G| * r_j
                gw_t = []
                for ic in range(NIC):
                    Gp = ps.tile([128, S], F32, tag="ps", name=f"G_{b}_{ic}")
                    for c in range(NCH):
                        nc.tensor.matmul(Gp[:], emb_t[c][:, ic * 128 : (ic + 1) * 128],
                                         emb_t[c][:], start=(c == 0), stop=(c == NCH - 1))
                    ga = gapool.tile([128, S], F32, tag="ga")
                    nc.scalar.activation(out=ga[:], in_=Gp[:], func=AFT.Abs,
                                         bias=0.0, scale=1.0)
                    gw = gwpool.tile([128, S], F32, tag="gw")
                    nc.vector.tensor_mul(gw[:], ga[:], W2[:])
                    gw_t.append(gw)

                # ---- exp + fused weighted reductions
                zwcols = tiny.tile([128, 2 * NIC], F32, tag="zwc")
                zcols = zwcols[:, 0:NIC]
                wcols = zwcols[:, NIC : 2 * NIC]
                for ic in range(NIC):
                    E = Epool.tile([128, S], F32, tag="E")
                    nc.scalar.activation(out=E[:], in_=L_ps[ic][:], func=AFT.Exp,
                                         bias=negm128[:], scale=1.0,
                                         accum_out=zcols[:, ic : ic + 1])
                    scr = scrpool.tile([128, S], F32, tag="scr")
                    nc.vector.scalar_tensor_tensor(
                        out=scr[:], in0=gw_t[ic][:], scalar=1.0, in1=E[:],
                        op0=ALU.mult, op1=ALU.mult,
                        accum_out=wcols[:, ic : ic + 1])

                nc.sync.dma_start(out=zw_d[b, 0], in_=zcols[:])
                nc.sync.dma_start(out=zw_d[b, 1], in_=wcols[:])

    nc.compile()
    _built = nc
    return nc


def kernel(embeddings, Wq, bq, Wk, bk, attention_masks, token_type_ids):
    global LAST_RESULTS
    nc = _build()

    embeddings = np.ascontiguousarray(np.asarray(embeddings, dtype=np.float32))
    Wq = np.asarray(Wq, dtype=np.float32)
    Wk = np.asarray(Wk, dtype=np.float32)
    bq = np.asarray(bq, dtype=np.float32)
    bk = np.asarray(bk, dtype=np.float32)
    am = np.asarray(attention_masks)
    tt = np.asarray(token_type_ids)

    # host-side layout + constant folding + fp32r rounding
    embT = _to_fp32r(embeddings.transpose(0, 2, 1)).reshape(B, NCH, 128, S)

    Wq64, Wk64 = Wq.astype(np.float64), Wk.astype(np.float64)
    A_aug = np.empty((DA, DA), np.float64)
    A_aug[:D, :D] = Wq64.T @ Wk64                  # A[d,d'] = sum_e Wq[e,d] Wk[e,d']
    A_aug[:D, D] = Wq64.T @ bk.astype(np.float64)   # u
    A_aug[D, :D] = Wk64.T @ bq.astype(np.float64)   # v
    A_aug[D, D] = float(bq.astype(np.float64) @ bk.astype(np.float64))
    AT = _to_fp32r(np.ascontiguousarray(A_aug.T).astype(np.float32))

    tok = am == 1
    m0 = tok & (tt == 0)
    m1 = tok & (tt == 1)
    m0neg = np.where(m0, np.float32(0.0), NEG).astype(np.float32)
    m1neg = np.where(m1, np.float32(0.0), NEG).astype(np.float32)
    ones_row = np.ones((B, 1, S), np.float32)
    lrows = _to_fp32r(np.concatenate([m0neg[:, None, :], ones_row, ones_row], axis=1))
    rrows = _to_fp32r(np.concatenate([ones_row, m1neg[:, None, :]], axis=1))
    ucol = np.ascontiguousarray(
        A_aug[:D, D].astype(np.float32).reshape(NCH, 128).T)        # [128, NCH]
    c0 = np.array([[A_aug[D, D]]], np.float32)

    in_maps = []
    for i in range(NCORES):
        sl = slice(i * BPC, (i + 1) * BPC)
        in_maps.append({
            "embT": np.ascontiguousarray(embT[sl]),
            "at": AT,
            "lrows": np.ascontiguousarray(lrows[sl]),
            "rrows": np.ascontiguousarray(rrows[sl]),
            "ones": np.ones((128, 1), np.float32),
            "onesrow": np.ones((1, S), np.float32),
            "ucol": ucol, "c0": c0,
        })

    res = run_bass_kernel_spmd(nc, in_maps, core_ids=list(range(NCORES)),
                               trace=PROFILE)
    LAST_RESULTS = res

    valid = m0.any(axis=1) & m1.any(axis=1)
    cs = np.zeros(B, np.float64)
    for i in range(NCORES):
        for j in range(BPC):
            b = i * BPC + j
            if not valid[b]:
                continue
            zcols = res.results[i]["zw"][j, 0].astype(np.float64)   # [128, NIC]
            wcols = res.results[i]["zw"][j, 1].astype(np.float64)
            r = res.results[i]["rout"][j].astype(np.float64)        # [S]
            ri = r.reshape(NIC, 128).T                              # [128, NIC]
            z = zcols.sum()
            w = (wcols * ri).sum()
            cs[b] = w / (z + 1e-30)
    return cs.astype(np.float32)



# revision 4
# speedup vs baseline: 2.2836x; 2.2836x over previous
"""Trainium2 Bass kernel for nn_CESAR_24309514895978 (ragged_sequence).

Math (per batch b):
  m0 = (attention_masks==1)&(token_type_ids==0); m1 = (attention_masks==1)&(token_type_ids==1)
  score[i,j] = |emb_n[i] . emb_n[j]|   (L2-normalized embeddings)
  logits[i,j] = (emb@Wq.T+bq)[i] . (emb@Wk.T+bk)[j]
  cs[b] = sum_{valid ij} softmax_flat(logits | pair_mask)[i,j] * score[i,j]

Ragged gather (host): only ~128 of 512 tokens are in each sentence, so the
host gathers sentence-0 tokens (q side, n0) and sentence-1 tokens (k side,
n1) per batch, padded with zeros to N slots.  All device matmuls then run on
[N x D] instead of [S x D]: ~3.7x fewer MACs than the dense form.

Constant folding (host, once):
  logits = embq @ A @ embk.T + uq[i] + prow[j],
  A = Wq.T@Wk,  uq = embq @ (Wq.T@bk),  prow = (Wk.T@bq) @ embk.T + bq.bk
uq rides the device exp() per-partition bias; exp(prow) (with zeros in k-pad
slots -- exact pad masking for free) and exp(prow)*rk ride as per-column
scale vectors.  Token norms r are computed exactly on host.

Device, per core (2 batches, bf16 matmuls, fp32 PSUM/elementwise):
  S1: Paug = A @ [embk_b0|embk_b1].T   (db-outer over 8 PSUM banks)
  S2 per batch, per i-chunk: L = embq.T.T @ Paug ; G = embq.T.T @ embkT
      (shared stationary), chunk-local max via DVE+gpsimd,
      E = exp(L - M_ic + uq) on ACT, z/w partials via DVE stt accum.
Host: combines chunk-local softmaxes exactly via the M_ic values, applies
r_i, and does the final tiny divisions in f64.
"""
import numpy as np
import ml_dtypes

import concourse.bass_isa as bass_isa
import concourse.tile as tile
from concourse import bacc, mybir
from concourse.bass_utils import run_bass_kernel_spmd

B, S, D = 16, 512, 1024
NCORES = 8
BPC = B // NCORES          # batches per core
NCH = D // 128             # 8 contraction chunks

F32 = mybir.dt.float32
BF16 = mybir.dt.bfloat16
AFT = mybir.ActivationFunctionType
ALU = mybir.AluOpType
AX = mybir.AxisListType

PROFILE = False            # set True (e.g. from test.py) to capture NTFF profile
LAST_RESULTS = None        # BassKernelResults of the last run (for test.py)

_built = {}


def _bf16(x: np.ndarray) -> np.ndarray:
    return np.ascontiguousarray(np.asarray(x, dtype=np.float32)).astype(
        ml_dtypes.bfloat16)


def _build(N: int):
    """Build the SPMD program for per-side pad size N (multiple of 32)."""
    if N in _built:
        return _built[N]

    W = BPC * N                       # concat width of the k/q token blocks
    NI = (N + 127) // 128             # i-chunks per batch
    icws = [min(128, N - 128 * ic) for ic in range(NI)]

    nc = bacc.Bacc("TRN2", target_bir_lowering=False, debug=False)

    embqT_d = nc.dram_tensor("embqT", [NCH, 128, W], BF16, kind="ExternalInput").ap()
    embkT_d = nc.dram_tensor("embkT", [NCH, 128, W], BF16, kind="ExternalInput").ap()
    at_d = nc.dram_tensor("at", [D, D], BF16, kind="ExternalInput").ap()
    # [b, 0, :] = exp(prow) (0 in pad slots), [b, 1, :] = exp(prow) * rk
    ep_d = nc.dram_tensor("eprows", [BPC, 2, N], F32, kind="ExternalInput").ap()
    uq_d = nc.dram_tensor("uq", [BPC, 128, NI], F32, kind="ExternalInput").ap()

    zw_d = nc.dram_tensor("zw", [BPC, 2, 128, NI], F32, kind="ExternalOutput").ap()
    mic_d = nc.dram_tensor("mic", [BPC, NI, 1, 1], F32, kind="ExternalOutput").ap()

    with tile.TileContext(nc) as tc:
        with (
            tc.tile_pool(name="apool", bufs=NCH) as apool,
            tc.tile_pool(name="ekpool", bufs=NCH) as ekpool,
            tc.tile_pool(name="eqpool", bufs=NCH) as eqpool,
            tc.tile_pool(name="ppool", bufs=NCH) as ppool,
            tc.tile_pool(name="wpool", bufs=2 * BPC) as wpool,
            tc.tile_pool(name="gpool", bufs=2) as gpool,
            tc.tile_pool(name="gwpool", bufs=2) as gwpool,
            tc.tile_pool(name="Epool", bufs=2) as Epool,
            tc.tile_pool(name="scrpool", bufs=2) as scrpool,
            tc.tile_pool(name="tiny", bufs=4) as tiny,
            tc.tile_pool(name="warmp", bufs=1) as warmp,
            tc.tile_pool(name="ps", bufs=8, space="PSUM") as ps,
        ):
            # ---- PE warm-up: busy-work on a memset tile while the first DMA
            # chunks land, so the HAM clock-gate releases before real matmuls.
            wsrc = warmp.tile([1, 64], BF16, tag="wsrc")
            nc.vector.memset(wsrc[:], 1.0)
            warm_ps = ps.tile([1, 64], F32, tag="ps", name="warm")
            for _ in range(24):
                nc.tensor.matmul(warm_ps[:], wsrc[:, 0:1], wsrc[:],
                                 start=True, stop=True)

            # ---- loads: S1 gate is at[db] + embk[db] in db order
            at_t, ek_t, eq_t = [], [], []
            for c in range(NCH):
                t = apool.tile([128, D], BF16, tag="a", name=f"at_{c}")
                nc.sync.dma_start(out=t[:], in_=at_d[c * 128:(c + 1) * 128, :])
                at_t.append(t)
                t = ekpool.tile([128, W], BF16, tag="ek", name=f"ek_{c}")
                nc.sync.dma_start(out=t[:], in_=embkT_d[c])
                ek_t.append(t)
            ept, uqt = [], []
            for b in range(BPC):
                rows = []
                for r in range(2):
                    t = tiny.tile([1, N], F32, tag="ep", name=f"ep_{b}_{r}")
                    nc.sync.dma_start(out=t[:], in_=ep_d[b, r:r + 1])
                    rows.append(t)
                ept.append(rows)
                t = tiny.tile([128, NI], F32, tag="uq", name=f"uq_{b}")
                nc.sync.dma_start(out=t[:], in_=uq_d[b])
                uqt.append(t)
            for c in range(NCH):
                t = eqpool.tile([128, W], BF16, tag="eq", name=f"eq_{c}")
                nc.sync.dma_start(out=t[:], in_=embqT_d[c])
                eq_t.append(t)

            # ---- per-batch column-scale broadcasts (gpsimd, early)
            EPb, WEb = [], []
            for b in range(BPC):
                t = wpool.tile([128, N], F32, tag="w2", name=f"EP_{b}")
                nc.gpsimd.partition_broadcast(t[:], ept[b][0][:], channels=128)
                EPb.append(t)
                t = wpool.tile([128, N], F32, tag="w2", name=f"WE_{b}")
                nc.gpsimd.partition_broadcast(t[:], ept[b][1][:], channels=128)
                WEb.append(t)

            # ---- S1: Paug = A @ embk_cat.T  (db-outer, 8 banks)
            st1 = [ps.tile([128, W], F32, tag="ps", name=f"st1_{da}")
                   for da in range(NCH)]
            for db in range(NCH):
                for da in range(NCH):
                    nc.tensor.matmul(st1[da][:],
                                     at_t[db][:, da * 128:(da + 1) * 128],
                                     ek_t[db][:],
                                     start=(db == 0), stop=(db == NCH - 1))
            paug = []
            for da in range(NCH):
                pt = ppool.tile([128, W], BF16, tag="paug", name=f"paug_{da}")
                if da % 2 == 0:
                    nc.scalar.copy(out=pt[:], in_=st1[da][:])
                else:
                    nc.vector.tensor_copy(pt[:], st1[da][:])
                paug.append(pt)

            # ---- S2 + gram per batch, per i-chunk (independent pipelines)
            for b in range(BPC):
                zwc = tiny.tile([128, 2 * NI], F32, tag="zwc", name=f"zw_{b}")
                zc = zwc[:, 0:NI]
                wc = zwc[:, NI:2 * NI]
                for ic in range(NI):
                    icw = icws[ic]
                    qo = b * N + ic * 128
                    Lp = ps.tile([icw, N], F32, tag="ps", name=f"L_{b}_{ic}")
                    Gp = ps.tile([icw, N], F32, tag="ps", name=f"G_{b}_{ic}")
                    for d in range(NCH):
                        nc.tensor.matmul(Lp[:], eq_t[d][:, qo:qo + icw],
                                         paug[d][:, b * N:(b + 1) * N],
                                         start=(d == 0), stop=(d == NCH - 1))
                        nc.tensor.matmul(Gp[:], eq_t[d][:, qo:qo + icw],
                                         ek_t[d][:, b * N:(b + 1) * N],
                                         start=(d == 0), stop=(d == NCH - 1))
                    # chunk-local max -> -M_ic (+uq) exp bias; M_ic to host
                    mx = tiny.tile([icw, 1], F32, tag="mx", bufs=2)
                    nc.vector.reduce_max(mx[:], Lp[:], axis=AX.X)
                    par = tiny.tile([icw, 1], F32, tag="par", bufs=2)
                    nc.gpsimd.partition_all_reduce(par[:], mx[:], channels=icw,
                                                   reduce_op=bass_isa.ReduceOp.max)
                    nc.sync.dma_start(out=mic_d[b, ic], in_=par[0:1, 0:1])
                    bias = tiny.tile([icw, 1], F32, tag="bias", bufs=2)
                    nc.vector.scalar_tensor_tensor(
                        out=bias[:], in0=par[:], scalar=-1.0,
                        in1=uqt[b][0:icw, ic:ic + 1],
                        op0=ALU.mult, op1=ALU.add)
                    # score weights: gw = |G| * exp(prow)*rk
                    ga = gpool.tile([icw, N], F32, tag="ga")
                    nc.scalar.activation(out=ga[:], in_=Gp[:], func=AFT.Abs,
                                         bias=0.0, scale=1.0)
                    gw = gwpool.tile([icw, N], F32, tag="gw")
                    nc.vector.tensor_mul(gw[:], ga[:], WEb[b][0:icw, :])
                    # E = exp(L - M_ic + uq); z/w partials
                    E = Epool.tile([icw, N], F32, tag="E")
                    nc.scalar.activation(out=E[:], in_=Lp[:], func=AFT.Exp,
                                         bias=bias[:], scale=1.0)
                    zscr = scrpool.tile([icw, N], F32, tag="scr")
                    nc.vector.scalar_tensor_tensor(
                        out=zscr[:], in0=E[:], scalar=1.0, in1=EPb[b][0:icw, :],
                        op0=ALU.mult, op1=ALU.mult,
                        accum_out=zc[0:icw, ic:ic + 1])
                    wscr = scrpool.tile([icw, N], F32, tag="scr")
                    nc.vector.scalar_tensor_tensor(
                        out=wscr[:], in0=E[:], scalar=1.0, in1=gw[:],
                        op0=ALU.mult, op1=ALU.mult,
                        accum_out=wc[0:icw, ic:ic + 1])
                nc.sync.dma_start(out=zw_d[b, 0], in_=zc[:])
                nc.sync.dma_start(out=zw_d[b, 1], in_=wc[:])

    nc.compile()
    _built[N] = nc
    return nc


def kernel(embeddings, Wq, bq, Wk, bk, attention_masks, token_type_ids):
    global LAST_RESULTS

    emb = np.ascontiguousarray(np.asarray(embeddings, dtype=np.float32))
    Wq64 = np.asarray(Wq, dtype=np.float64)
    Wk64 = np.asarray(Wk, dtype=np.float64)
    bq64 = np.asarray(bq, dtype=np.float64)
    bk64 = np.asarray(bk, dtype=np.float64)
    am = np.asarray(attention_masks)
    tt = np.asarray(token_type_ids)

    tok = am == 1
    m0 = tok & (tt == 0)
    m1 = tok & (tt == 1)
    n0 = m0.sum(1)
    n1 = m1.sum(1)
    nmax = max(int(n0.max()), int(n1.max()), 32)
    N = ((nmax + 31) // 32) * 32
    NI = (N + 127) // 128

    nc = _build(N)

    # host-side constant folding (f64)
    A = Wq64.T @ Wk64
    u = Wq64.T @ bk64
    v = Wk64.T @ bq64
    c0 = float(bq64 @ bk64)
    AT = _bf16(A.T)

    emb64 = emb.astype(np.float64)
    in_maps = []
    rq_all, idx0_all = [], []
    for core in range(NCORES):
        embqT = np.zeros((NCH, 128, BPC * N), ml_dtypes.bfloat16)
        embkT = np.zeros((NCH, 128, BPC * N), ml_dtypes.bfloat16)
        eprows = np.zeros((BPC, 2, N), np.float32)
        uqa = np.zeros((BPC, 128, NI), np.float32)
        for b in range(BPC):
            g = core * BPC + b
            eq = emb64[g][m0[g]]                      # [n0, D]
            ek = emb64[g][m1[g]]                      # [n1, D]
            k0, k1 = eq.shape[0], ek.shape[0]
            embqT[:, :, b * N:b * N + k0] = \
                _bf16(eq.T).reshape(NCH, 128, k0)
            embkT[:, :, b * N:b * N + k1] = \
                _bf16(ek.T).reshape(NCH, 128, k1)
            prow = v @ ek.T + c0                      # [n1]
            rk = 1.0 / np.maximum(np.sqrt((ek * ek).sum(1)), 1e-12)
            ep = np.exp(prow)
            eprows[b, 0, :k1] = ep
            eprows[b, 1, :k1] = ep * rk
            uq = (eq @ u).astype(np.float32)          # [n0]
            uqp = np.zeros(NI * 128, np.float32)
            uqp[:k0] = uq
            uqa[b] = uqp.reshape(NI, 128).T
            rq_all.append(1.0 / np.maximum(np.sqrt((eq * eq).sum(1)), 1e-12))
            idx0_all.append(k0)
        in_maps.append({
            "embqT": embqT, "embkT": embkT, "at": AT,
            "eprows": eprows, "uq": uqa,
        })

    res = run_bass_kernel_spmd(nc, in_maps, core_ids=list(range(NCORES)),
                               trace=PROFILE)
    LAST_RESULTS = res

    cs = np.zeros(B, np.float64)
    for core in range(NCORES):
        for b in range(BPC):
            g = core * BPC + b
            k0 = idx0_all[g]
            if k0 == 0 or int(n1[g]) == 0:
                continue
            zcols = res.results[core]["zw"][b, 0].astype(np.float64)  # [128, NI]
            wcols = res.results[core]["zw"][b, 1].astype(np.float64)
            mic = res.results[core]["mic"][b].reshape(NI).astype(np.float64)
            rq = rq_all[g]
            z = w = 0.0
            Mg = mic[: (k0 + 127) // 128].max()
            for ic in range(NI):
                icw = min(128, max(0, k0 - ic * 128))
                if icw == 0:
                    break
                f = np.exp(mic[ic] - Mg)
                z += zcols[:icw, ic].sum() * f
                w += (wcols[:icw, ic] * rq[ic * 128:ic * 128 + icw]).sum() * f
            cs[g] = w / (z + 1e-30)
    return cs.astype(np.float32)


# revision 6
# speedup vs baseline: 3.0039x; 1.3154x over previous
"""Trainium2 Bass kernel for nn_CESAR_24309514895978 (ragged_sequence).

Math (per batch b):
  m0 = (attention_masks==1)&(token_type_ids==0); m1 = (attention_masks==1)&(token_type_ids==1)
  score[i,j] = |emb_n[i] . emb_n[j]|   (L2-normalized embeddings)
  logits[i,j] = (emb@Wq.T+bq)[i] . (emb@Wk.T+bk)[j]
  cs[b] = sum_{valid ij} softmax_flat(logits | pair_mask)[i,j] * score[i,j]

Ragged gather (host): only ~128 of 512 tokens are in each sentence, so the
host gathers sentence-0 tokens (q side) and sentence-1 tokens (k side) per
batch, zero-padded to N slots.  Device matmuls run on [N x D] instead of
[S x D]: ~3.7x fewer MACs than the dense form, in bf16 (fp32 PSUM).

Constant folding (host, once):
  logits = embq @ A @ embk.T + uq[i] + prow[j],
  A = Wq.T@Wk,  uq = embq @ (Wq.T@bk),  prow = (Wk.T@bq) @ embk.T + bq.bk
uq rides the device exp() per-partition bias; exp(prow) (0 in k-pad slots --
exact pad masking for free) and exp(prow)*r_k ride as host-replicated
per-column scale planes.  Token norms r are computed exactly on host.

Device layout: ONE SBUF mega-tile [128, 2, HALF].  half0 is filled by 9 big
DMAs ([at_db | ek_db] groups so stage-1 starts after the first 0.34MB, then
eq+scales); half1 holds Paug at the same offsets as ek.  A single fused S2
matmul per (d, batch, i-chunk) then streams [ek | Paug] through shared
stationary eq weights, yielding gram and logit chunks in one PSUM tile:
  S1: Paug = A @ [ek_b0|ek_b1].T    (db-outer over 8 PSUM banks)
  S2: [G | L] = eq_ic.T.T @ [ek_b | Paug_b];  chunk-local max (DVE+gpsimd),
      E = exp(L - M_ic + uq) on ACT, z/w partials via DVE stt accum.
Host: combines chunk-local softmaxes exactly via the M_ic values (shipped
through a column of the single output tile), applies r_i, divides in f64.
"""
import numpy as np
import ml_dtypes

import concourse.bass_isa as bass_isa
import concourse.tile as tile
from concourse import bacc, mybir
from concourse.bass_utils import run_bass_kernel_spmd

B, S, D = 16, 512, 1024
NCORES = 8
BPC = B // NCORES          # batches per core
NCH = D // 128             # 8 contraction chunks

F32 = mybir.dt.float32
BF16 = mybir.dt.bfloat16
AFT = mybir.ActivationFunctionType
ALU = mybir.AluOpType
AX = mybir.AxisListType

PROFILE = False            # set True (e.g. from test.py) to capture NTFF profile
LAST_RESULTS = None        # BassKernelResults of the last run (for test.py)
NWARM = 100                # PE warm-up matmuls issued while the first DMA lands

_built = {}


def _bf16(x: np.ndarray) -> np.ndarray:
    return np.ascontiguousarray(np.asarray(x, dtype=np.float32)).astype(
        ml_dtypes.bfloat16)


def _build(N: int):
    """Build the SPMD program for per-side pad size N (multiple of 32)."""
    if N in _built:
        return _built[N]

    W = BPC * N                  # concat width of the k/q token blocks
    NI = (N + 127) // 128        # i-chunks per batch
    icws = [min(128, N - 128 * ic) for ic in range(NI)]
    G = D + W                    # [at_db | ek_db] group width (bf16 cols)
    EQo = NCH * G
    EPo = EQo + NCH * W
    UQo = EPo + BPC * 2 * 2 * N
    TOT = UQo + BPC * 2 * NI

    nc = bacc.Bacc("TRN2", target_bir_lowering=False, debug=False)

    mi_d = nc.dram_tensor("mi", [128, TOT], BF16, kind="ExternalInput").ap()
    zw_d = nc.dram_tensor("zw", [128, BPC * 3 * NI], F32,
                          kind="ExternalOutput").ap()

    with tile.TileContext(nc) as tc:
        with (
            tc.tile_pool(name="mega", bufs=1) as megapool,
            tc.tile_pool(name="gpool", bufs=2) as gpool,
            tc.tile_pool(name="gwpool", bufs=2) as gwpool,
            tc.tile_pool(name="Epool", bufs=2) as Epool,
            tc.tile_pool(name="scrpool", bufs=2) as scrpool,
            tc.tile_pool(name="tiny", bufs=4) as tiny,
            tc.tile_pool(name="warmp", bufs=1) as warmp,
            tc.tile_pool(name="ps", bufs=8, space="PSUM") as ps,
        ):
            # ---- PE warm-up: busy-work on a memset tile while the first DMA
            # groups land, so the HAM clock-gate is released for real matmuls.
            wsrc = warmp.tile([1, 64], BF16, tag="wsrc")
            nc.vector.memset(wsrc[:], 1.0)
            warm_ps = ps.tile([1, 64], F32, tag="ps", name="warm")
            for _ in range(NWARM):
                nc.tensor.matmul(warm_ps[:], wsrc[:, 0:1], wsrc[:],
                                 start=True, stop=True)

            mega = megapool.tile([128, 2, TOT], BF16, tag="mega")
            # 8 gating DMA groups [at_db | ek_db], then the eq/scales tail
            for db in range(NCH):
                nc.sync.dma_start(out=mega[:, 0, db * G:(db + 1) * G],
                                  in_=mi_d[:, db * G:(db + 1) * G])
            nc.sync.dma_start(out=mega[:, 0, EQo:TOT], in_=mi_d[:, EQo:TOT])

            def at_ap(db, da):
                return mega[:, 0, db * G + da * 128: db * G + (da + 1) * 128]

            def ek_ap(db):                      # S1 moving, both batches
                return mega[:, 0, db * G + D: db * G + D + W]

            def ekpaug_ap(d, b):                # S2 moving [ek_b | paug_b]
                o = d * G + D + b * N
                return mega[:, :, o:o + N]

            def paug_ap(da, b):                 # S1 copy destination
                o = da * G + D + b * N
                return mega[:, 1, o:o + N]

            def eq_ap(d, b, ic, icw):           # S2 stationary
                o = EQo + d * W + b * N + ic * 128
                return mega[:, 0, o:o + icw]

            def ep_ap(b):                       # exp(prow) plane, f32
                o = EPo + (b * 2) * 2 * N
                return mega[:, 0, o:o + 2 * N].bitcast(F32)

            def we_ap(b):                       # exp(prow)*rk plane, f32
                o = EPo + (b * 2 + 1) * 2 * N
                return mega[:, 0, o:o + 2 * N].bitcast(F32)

            def uq_ap(b):                       # [128, NI] f32
                o = UQo + b * 2 * NI
                return mega[:, 0, o:o + 2 * NI].bitcast(F32)

            # ---- S1: Paug = A @ ek_cat.T  (db-outer, 8 banks)
            st1 = [ps.tile([128, W], F32, tag="ps", name=f"st1_{da}")
                   for da in range(NCH)]
            for db in range(NCH):
                for da in range(NCH):
                    nc.tensor.matmul(st1[da][:], at_ap(db, da), ek_ap(db),
                                     start=(db == 0), stop=(db == NCH - 1))
            for b in range(BPC):
                for da in range(NCH):
                    if da % 2 == 0:
                        nc.scalar.copy(out=paug_ap(da, b),
                                       in_=st1[da][:, b * N:(b + 1) * N])
                    else:
                        nc.vector.tensor_copy(paug_ap(da, b),
                                              st1[da][:, b * N:(b + 1) * N])

            # ---- S2: fused [G | L] per (batch, i-chunk); local softmax stats
            zwall = tiny.tile([128, BPC * 3 * NI], F32, tag="zwall")
            for b in range(BPC):
                zo = b * 3 * NI
                for ic in range(NI):
                    icw = icws[ic]
                    LG = ps.tile([icw, 2 * N], F32, tag="ps",
                                 name=f"LG_{b}_{ic}")
                    for d in range(NCH):
                        nc.tensor.matmul(LG[:], eq_ap(d, b, ic, icw),
                                         ekpaug_ap(d, b),
                                         start=(d == 0), stop=(d == NCH - 1))
                    Gp = LG[:, 0:N]
                    Lp = LG[:, N:2 * N]
                    mx = tiny.tile([icw, 1], F32, tag="mx", bufs=2)
                    nc.vector.reduce_max(mx[:], Lp, axis=AX.X)
                    par = tiny.tile([icw, 1], F32, tag="par", bufs=2)
                    nc.gpsimd.partition_all_reduce(
                        par[:], mx[:], channels=icw,
                        reduce_op=bass_isa.ReduceOp.max)
                    nc.vector.tensor_copy(
                        zwall[0:icw, zo + 2 * NI + ic:zo + 2 * NI + ic + 1],
                        par[:])
                    bias = tiny.tile([icw, 1], F32, tag="bias", bufs=2)
                    nc.vector.scalar_tensor_tensor(
                        out=bias[:], in0=par[:], scalar=-1.0,
                        in1=uq_ap(b)[0:icw, ic:ic + 1],
                        op0=ALU.mult, op1=ALU.add)
                    ga = gpool.tile([icw, N], F32, tag="ga")
                    nc.scalar.activation(out=ga[:], in_=Gp, func=AFT.Abs,
                                         bias=0.0, scale=1.0)
                    gw = gwpool.tile([icw, N], F32, tag="gw")
                    nc.vector.tensor_mul(gw[:], ga[:], we_ap(b)[0:icw, :])
                    E = Epool.tile([icw, N], F32, tag="E")
                    nc.scalar.activation(out=E[:], in_=Lp, func=AFT.Exp,
                                         bias=bias[:], scale=1.0)
                    zscr = scrpool.tile([icw, N], F32, tag="scr")
                    nc.vector.scalar_tensor_tensor(
                        out=zscr[:], in0=E[:], scalar=1.0,
                        in1=ep_ap(b)[0:icw, :],
                        op0=ALU.mult, op1=ALU.mult,
                        accum_out=zwall[0:icw, zo + ic:zo + ic + 1])
                    wscr = scrpool.tile([icw, N], F32, tag="scr")
                    nc.vector.scalar_tensor_tensor(
                        out=wscr[:], in0=E[:], scalar=1.0, in1=gw[:],
                        op0=ALU.mult, op1=ALU.mult,
                        accum_out=zwall[0:icw, zo + NI + ic:zo + NI + ic + 1])
            nc.sync.dma_start(out=zw_d, in_=zwall[:])

    nc.compile()
    _built[N] = (nc, G, EQo, EPo, UQo, TOT, NI)
    return _built[N]


def kernel(embeddings, Wq, bq, Wk, bk, attention_masks, token_type_ids):
    global LAST_RESULTS

    emb = np.ascontiguousarray(np.asarray(embeddings, dtype=np.float32))
    Wq64 = np.asarray(Wq, dtype=np.float64)
    Wk64 = np.asarray(Wk, dtype=np.float64)
    bq64 = np.asarray(bq, dtype=np.float64)
    bk64 = np.asarray(bk, dtype=np.float64)
    am = np.asarray(attention_masks)
    tt = np.asarray(token_type_ids)

    tok = am == 1
    m0 = tok & (tt == 0)
    m1 = tok & (tt == 1)
    n0 = m0.sum(1)
    n1 = m1.sum(1)
    nmax = max(int(n0.max()), int(n1.max()), 32)
    N = ((nmax + 31) // 32) * 32

    nc, G, EQo, EPo, UQo, TOT, NI = _build(N)
    W = BPC * N

    # host-side constant folding (f64)
    A = Wq64.T @ Wk64
    u = Wq64.T @ bk64
    v = Wk64.T @ bq64
    c0 = float(bq64 @ bk64)
    ATr = _bf16(A.T).reshape(NCH, 128, D)

    emb64 = emb.astype(np.float64)
    in_maps = []
    rq_all, k0_all = [], []
    for core in range(NCORES):
        mi = np.zeros((128, TOT), ml_dtypes.bfloat16)
        epwe = np.zeros((BPC * 2, N), np.float32)
        uqa = np.zeros((BPC, 128, NI), np.float32)
        embqT = np.zeros((NCH, 128, W), ml_dtypes.bfloat16)
        for db in range(NCH):
            mi[:, db * G:db * G + D] = ATr[db]
        for b in range(BPC):
            g = core * BPC + b
            eq = emb64[g][m0[g]]                      # [n0, D]
            ek = emb64[g][m1[g]]                      # [n1, D]
            k0, k1 = eq.shape[0], ek.shape[0]
            ekT = _bf16(ek.T).reshape(NCH, 128, k1)
            for db in range(NCH):
                mi[:, db * G + D + b * N: db * G + D + b * N + k1] = ekT[db]
            embqT[:, :, b * N:b * N + k0] = _bf16(eq.T).reshape(NCH, 128, k0)
            prow = v @ ek.T + c0                      # [n1]
            rk = 1.0 / np.maximum(np.sqrt((ek * ek).sum(1)), 1e-12)
            ep = np.exp(prow)
            epwe[b * 2, :k1] = ep
            epwe[b * 2 + 1, :k1] = ep * rk
            uq = (eq @ u).astype(np.float32)          # [n0]
            uqp = np.zeros(NI * 128, np.float32)
            uqp[:k0] = uq
            uqa[b] = uqp.reshape(NI, 128).T
            rq_all.append(1.0 / np.maximum(np.sqrt((eq * eq).sum(1)), 1e-12))
            k0_all.append(k0)
        mi[:, EQo:EPo] = embqT.transpose(1, 0, 2).reshape(128, NCH * W)
        mi[:, EPo:UQo] = np.ascontiguousarray(np.broadcast_to(
            epwe.reshape(1, BPC * 2 * N), (128, BPC * 2 * N)
        )).view(ml_dtypes.bfloat16)
        mi[:, UQo:TOT] = np.ascontiguousarray(
            uqa.transpose(1, 0, 2)).reshape(128, BPC * NI
                                            ).view(ml_dtypes.bfloat16)
        in_maps.append({"mi": mi})

    res = run_bass_kernel_spmd(nc, in_maps, core_ids=list(range(NCORES)),
                               trace=PROFILE)
    LAST_RESULTS = res

    cs = np.zeros(B, np.float64)
    for core in range(NCORES):
        zw = res.results[core]["zw"].astype(np.float64)  # [128, BPC*3*NI]
        for b in range(BPC):
            g = core * BPC + b
            k0 = k0_all[g]
            if k0 == 0 or int(n1[g]) == 0:
                continue
            zo = b * 3 * NI
            rq = rq_all[g]
            nic = (k0 + 127) // 128
            mic = np.array([zw[0, zo + 2 * NI + ic] for ic in range(nic)])
            Mg = mic.max()
            z = w = 0.0
            for ic in range(nic):
                icw = min(128, k0 - ic * 128)
                f = np.exp(mic[ic] - Mg)
                z += zw[:icw, zo + ic].sum() * f
                w += (zw[:icw, zo + NI + ic]
                      * rq[ic * 128:ic * 128 + icw]).sum() * f
            cs[g] = w / (z + 1e-30)
    return cs.astype(np.float32)
